# revision 9
# baseline (speedup 1.0000x reference)
"""Chamfer loss kernel for Trainium2 (8 NeuronCores, SPMD data-parallel over batch).

Math: the device computes s = -d2 where d2[n, m] = |p_n|^2 + |g_m|^2 - 2 p_n.g_m,
so every min the loss needs becomes a max on device. s is produced directly by
an augmented matmul on the PE: every fp32 operand is split into three bf16
terms (h + m + l); retaining the product pairs hh, hm, mh, hl, lh, mm
reproduces each fp32 product to ~2^-27 rel. With 3 coords x 6 pairs + 3 |p|^2
rows + 3 |g|^2 rows the contraction dim is K=24, all bf16, accumulated into
fp32 PSUM.

Per [128, 2048] PSUM chunk, ScalarE stages s into SBUF as bf16 (one rounding,
~2^-9 rel — harmless at 2e-2 tolerance). That unlocks the DVE 2x_1p fast mode
(packed 2-byte operands) for tensor_tensor, which tensor_reduce never gets:

- Row (pred-point) max: per p, one tensor_tensor(max) folds the two staged g
  chunks into a [128, 2048] scratch, then log2 folds by contiguous halves at
  2x down to a 128-wide stub in rowpart. One batch-final tensor_reduce folds
  rowpart [128, 4096] -> rowacc [128, 32].
- Column (gt-point) max: running colstate[g] = max(colstate, chunk) via
  tensor_tensor(max) at 2x. The p==0 chunk is staged by ScalarE directly into
  colstate[g] (initializing it for free).
- GpSimd relief: for GP_PS p-chunks both g-chunks' column pass runs as a
  direct partition_all_reduce(max) on GpSimd instead of the DVE colstate op;
  partials DMA to DRAM and the host max-folds them. These chunks stage into a
  dedicated gp_pool so ScalarE's in-order queue never waits on GpSimd's slow
  (7.7us) tile reads — that coupling stalled DVE ~3.7us per GpSimd op in v2.
  GpSimd also reduces the 2 colstate finals.

sqrt + means run on the host (min/max commute with sqrt/clamp after negation).
Each core handles 4 of the 32 batches. No collectives; host combines scalars.
"""

import sys

for _p in ("/opt/trn_rl_repo",):
    if _p not in sys.path:
        sys.path.insert(0, _p)

from contextlib import ExitStack
from functools import lru_cache

import ml_dtypes
import numpy as np

import concourse.bass as bass
import concourse.tile as tile
from concourse import bacc, bass_isa, mybir
from concourse.bass_utils import run_bass_kernel_spmd

F32 = mybir.dt.float32
BF16 = mybir.dt.bfloat16
MAX = mybir.AluOpType.max
NPBF16 = ml_dtypes.bfloat16

B, N, M = 32, 4096, 4096
NCORES = 8
BPC = B // NCORES  # batches per core
K = 24             # augmented contraction dim (3 coords x 6 bf16 pairs + 2x3 norm rows)
PCH = 128          # pred chunk size (PE partitions)
NP = N // PCH      # 32 pred chunks
FDV = 2048         # chunk free size (4 PSUM banks)
NG = M // FDV      # gt chunks per batch row pass
MMN = 512          # matmul moving free dim (one fp32 PSUM bank)
SEG = 128          # row-fold stub width left for the batch-final reduce
# p-chunks whose column pass runs on GpSimd (both g) instead of DVE colstate.
GP_PS = (2, 6, 10, 14, 18, 22, 26)
NGP = len(GP_PS)   # gp rows per (batch, g) group


def _build_program():
    nc = bacc.Bacc(
        "TRN2", target_bir_lowering=False, debug=False, num_devices=NCORES
    )
    lhs = nc.dram_tensor("lhs", [BPC * K, N], BF16, kind="ExternalInput").ap()
    rhs = nc.dram_tensor("rhs", [BPC * K, M], BF16, kind="ExternalInput").ap()
    rowmin = nc.dram_tensor("rowmin", [BPC * PCH, NP], F32, kind="ExternalOutput").ap()
    # per (batch, g-chunk): row 0 = colstate final; rows 1..NGP = GpSimd
    # per-chunk column partials. Host max-combines.
    colmin = nc.dram_tensor(
        "colmin", [BPC * NG * (NGP + 1), FDV], BF16, kind="ExternalOutput"
    ).ap()

    with tile.TileContext(nc) as tc, ExitStack() as ctx:
        lr_pool = ctx.enter_context(tc.tile_pool(name="lr", bufs=2))
        col_pool = ctx.enter_context(tc.tile_pool(name="col", bufs=2 * NG))
        red_pool = ctx.enter_context(tc.tile_pool(name="red", bufs=8))
        d2_pool = ctx.enter_context(tc.tile_pool(name="d2", bufs=8))
        gp_pool = ctx.enter_context(tc.tile_pool(name="gp", bufs=8))
        scr_pool = ctx.enter_context(tc.tile_pool(name="scr", bufs=4))
        acc_pool = ctx.enter_context(tc.tile_pool(name="acc", bufs=2))
        rp_pool = ctx.enter_context(tc.tile_pool(name="rp", bufs=2))
        psum_pool = ctx.enter_context(tc.tile_pool(name="psum", bufs=2, space="PSUM"))

        for i in range(BPC):
            L = lr_pool.tile([K, N], BF16, tag="L")
            nc.sync.dma_start(L[:], lhs[K * i : K * (i + 1), :])
            R = lr_pool.tile([K, M], BF16, tag="R")
            nc.sync.dma_start(R[:], rhs[K * i : K * (i + 1), :])

            colstate = [
                col_pool.tile([PCH, FDV], BF16, tag="cs", name=f"cs_{i}_{g}")
                for g in range(NG)
            ]
            rowpart = rp_pool.tile([PCH, NP * SEG], BF16, tag="rowpart")
            rowacc = acc_pool.tile([PCH, NP], F32, tag="rowacc")

            for p in range(NP):
                # scr layout: [0:1024) fold1(g0), [1024:2048) fold1(g1),
                # [2048:3072) combine, then the fold ladder segments
                # [3072:3584), [3584:3840); final SEG fold lands in rowpart.
                scr = scr_pool.tile([PCH, 2 * FDV], BF16, tag="scr")
                for g in range(NG):
                    # s = -d2 for this [PCH, FDV] chunk, via augmented matmul
                    ps = psum_pool.tile([PCH, FDV], F32, tag="ps")
                    for s in range(FDV // MMN):
                        nc.tensor.matmul(
                            ps[:, MMN * s : MMN * (s + 1)],
                            lhsT=L[:, PCH * p : PCH * (p + 1)],
                            rhs=R[:, FDV * g + MMN * s : FDV * g + MMN * (s + 1)],
                            start=True,
                            stop=True,
                        )
                    # stage to SBUF as bf16; p==0 lands directly in colstate,
                    # GpSimd chunks land in their own pool (see module doc)
                    if p == 0:
                        dst = colstate[g]
                    elif p in GP_PS:
                        dst = gp_pool.tile([PCH, FDV], BF16, tag="gp")
                    else:
                        dst = d2_pool.tile([PCH, FDV], BF16, tag="d2")
                    nc.scalar.copy(dst[:], ps[:])

                    # column pass
                    if p == 0:
                        pass  # colstate initialized by the staging copy
                    elif p in GP_PS:
                        csr = red_pool.tile(
                            [PCH, FDV], BF16, tag="csr", name=f"gp_{i}_{p}_{g}"
                        )
                        nc.gpsimd.partition_all_reduce(
                            csr[:], dst[:], channels=PCH,
                            reduce_op=bass_isa.ReduceOp.max,
                        )
                        row = (i * NG + g) * (NGP + 1) + 1 + GP_PS.index(p)
                        nc.sync.dma_start(colmin[row : row + 1, :], csr[0:1, :])
                    else:
                        nc.vector.tensor_tensor(
                            out=colstate[g][:], in0=colstate[g][:], in1=dst[:], op=MAX
                        )

                    # row pass level 1: fold this chunk 2048 -> 1024 now, so
                    # DVE never needs more than the chunk Act just staged
                    # (a 2-chunk demand jump at GpSimd p's stalled v3 ~3.9us)
                    nc.vector.tensor_tensor(
                        out=scr[:, g * (FDV // 2) : (g + 1) * (FDV // 2)],
                        in0=dst[:, 0 : FDV // 2],
                        in1=dst[:, FDV // 2 : FDV],
                        op=MAX,
                    )

                # row pass: combine the two folded halves, then finish the
                # log2 ladder into successive scr segments
                src_off, w, pos = 0, FDV // 2, FDV
                while w >= SEG:
                    out_ap = (
                        rowpart[:, p * SEG : (p + 1) * SEG]
                        if w == SEG
                        else scr[:, pos : pos + w]
                    )
                    nc.vector.tensor_tensor(
                        out=out_ap,
                        in0=scr[:, src_off : src_off + w],
                        in1=scr[:, src_off + w : src_off + 2 * w],
                        op=MAX,
                    )
                    src_off = pos
                    pos += w
                    w //= 2

            # batch-final row fold: all NP SEG-wide stubs -> [PCH, NP]
            nc.vector.tensor_reduce(
                out=rowacc[:],
                in_=rowpart[:].rearrange("p (a b) -> p a b", b=SEG),
                axis=mybir.AxisListType.X,
                op=MAX,
            )
            nc.sync.dma_start(rowmin[PCH * i : PCH * (i + 1), :], rowacc[:])

            # colstate finals on GpSimd
            for g in range(NG):
                csr = red_pool.tile([PCH, FDV], BF16, tag="csr", name=f"csr_{i}_{g}")
                nc.gpsimd.partition_all_reduce(
                    csr[:], colstate[g][:], channels=PCH,
                    reduce_op=bass_isa.ReduceOp.max,
                )
                row = (i * NG + g) * (NGP + 1)
                nc.sync.dma_start(colmin[row : row + 1, :], csr[0:1, :])

    nc.compile()
    return nc


@lru_cache(maxsize=1)
def _get_program():
    return _build_program()


def _split3(x):
    """fp32 -> three bf16 terms whose sum matches x to ~2^-27 rel."""
    h = x.astype(NPBF16)
    r = x - h.astype(np.float32)
    m = r.astype(NPBF16)
    l = (r - m.astype(np.float32)).astype(NPBF16)
    return h, m, l


def _make_inputs(pred, gt):
    """Host-side packing of the K=24 bf16 split operands (for -d2), per core."""
    pred = np.ascontiguousarray(pred, dtype=np.float32)
    gt = np.ascontiguousarray(gt, dtype=np.float32)
    p2 = np.einsum("bnd,bnd->bn", pred, pred)
    g2 = np.einsum("bmd,bmd->bm", gt, gt)
    Lr, Rr = [], []
    for d in range(3):
        u = np.float32(2.0) * pred[:, :, d]  # +2 so the dot yields -d2
        v = gt[:, :, d]
        uh, um, ul = _split3(u)
        vh, vm, vl = _split3(v)
        # product pairs kept: hh, hm, mh, hl, lh, mm
        Lr += [uh, uh, um, uh, ul, um]
        Rr += [vh, vm, vh, vl, vh, vm]
    ph, pm, pl = _split3(-p2)
    gh, gm, gl = _split3(g2)
    ones_n = np.ones_like(p2, dtype=NPBF16)
    neg_n = -ones_n
    ones_m = np.ones_like(g2, dtype=NPBF16)
    Lr += [ph, pm, pl, neg_n, neg_n, neg_n]
    Rr += [ones_m, ones_m, ones_m, gh, gm, gl]
    lhs = np.stack(Lr, axis=1)  # [B, K, N] bf16
    rhs = np.stack(Rr, axis=1)  # [B, K, M] bf16
    in_maps = []
    for c in range(NCORES):
        sl = slice(c * BPC, (c + 1) * BPC)
        in_maps.append(
            {
                "lhs": np.ascontiguousarray(lhs[sl].reshape(BPC * K, N)),
                "rhs": np.ascontiguousarray(rhs[sl].reshape(BPC * K, M)),
            }
        )
    return in_maps


def _finish(results):
    # device values are maxes of -d2: negate back to d2 mins
    rowmins = -np.stack([r["rowmin"] for r in results]).astype(np.float64)
    colraw = np.stack(
        [np.asarray(r["colmin"]).astype(np.float32) for r in results]
    )  # [8, BPC*NG*(NGP+1), FDV]
    colmins = -(
        colraw.reshape(NCORES, BPC, NG, NGP + 1, FDV).max(axis=3)
    ).reshape(NCORES, BPC, M).astype(np.float64)
    ch2 = np.sqrt(np.maximum(rowmins, 1e-12)).mean()
    ch1 = np.sqrt(np.maximum(colmins, 1e-12)).mean()
    return np.asarray(ch1 + ch2, dtype=np.float32)


def kernel(pred, gt):
    nc = _get_program()
    in_maps = _make_inputs(pred, gt)
    res = run_bass_kernel_spmd(nc, in_maps, list(range(NCORES)))
    return _finish(res.results)


if __name__ == "__main__":
    rng = np.random.default_rng(0)
    pred = rng.standard_normal((B, N, 3), dtype=np.float32)
    gt = rng.standard_normal((B, M, 3), dtype=np.float32)
    print(kernel(pred, gt))


# revision 12
# speedup vs baseline: 1.0794x; 1.0794x over previous
"""Chamfer loss kernel for Trainium2 (8 NeuronCores, SPMD data-parallel over batch).

Math: the device computes s = -d2 where d2[n, m] = |p_n|^2 + |g_m|^2 - 2 p_n.g_m,
so every min the loss needs becomes a max on device. s is produced directly by
an augmented matmul on the PE: every fp32 operand is split into three bf16
terms (h + m + l); retaining the product pairs hh, hm, mh, hl, lh, mm
reproduces each fp32 product to ~2^-27 rel. With 3 coords x 6 pairs + 3 |p|^2
rows + 3 |g|^2 rows the contraction dim is K=24, all bf16, accumulated into
fp32 PSUM.

Per [128, 2048] PSUM chunk, ScalarE stages s into SBUF as bf16 (one rounding,
~2^-9 rel — harmless at 2e-2 tolerance). That unlocks the DVE 2x_1p fast mode
(packed 2-byte operands) for tensor_tensor, which tensor_reduce never gets.
All reductions are built from tensor_tensor(max) at 2x:

- Column (gt-point) max: running colstate[g] = max(colstate, chunk). The p==0
  chunk is staged by ScalarE directly into colstate[g] (initializing it).
- Row (pred-point) max: per p, combine the two staged g chunks ([128,2048]
  out), then a log2 fold ladder by contiguous halves down to a 128-wide stub
  in rowpart; a per-batch ladder folds rowpart [128,4096] -> rowacc [128,32].
- GpSimd only reduces the 2 colstate finals per batch (partition_all_reduce).
  Earlier revisions offloaded some chunks' column pass to GpSimd; the
  event-semaphore coupling (Act waits Pool/DVE, DVE waits Act) serialized the
  pipeline around every GpSimd op and cost far more than the offload saved.

sqrt + means run on the host (min/max commute with sqrt/clamp after negation).
Each core handles 4 of the 32 batches. No collectives; host combines scalars.
"""

import sys

for _p in ("/opt/trn_rl_repo",):
    if _p not in sys.path:
        sys.path.insert(0, _p)

from contextlib import ExitStack
from functools import lru_cache

import ml_dtypes
import numpy as np

import concourse.bass as bass
import concourse.tile as tile
from concourse import bacc, bass_isa, mybir
from concourse.bass_utils import run_bass_kernel_spmd

F32 = mybir.dt.float32
BF16 = mybir.dt.bfloat16
MAX = mybir.AluOpType.max
NPBF16 = ml_dtypes.bfloat16

B, N, M = 32, 4096, 4096
NCORES = 8
BPC = B // NCORES  # batches per core
K = 24             # augmented contraction dim (3 coords x 6 bf16 pairs + 2x3 norm rows)
PCH = 128          # pred chunk size (PE partitions)
NP = N // PCH      # 32 pred chunks
FDV = 2048         # chunk free size (4 PSUM banks)
NG = M // FDV      # gt chunks per batch row pass
MMN = 512          # matmul moving free dim (one fp32 PSUM bank)
SEG = 128          # row-fold stub width left for the batch-final ladder
# p-chunks whose column pass runs on GpSimd (both g) instead of DVE
# colstate. Ring depths below make every WAR wait on these reference
# last-batch GpSimd work (fresh Pool waits on the ScalarE queue
# serialized the pipeline in earlier revisions).
GP_PS = (16, 20, 24, 28)
NGP = len(GP_PS)   # gp rows per (batch, g) group


def _build_program():
    nc = bacc.Bacc(
        "TRN2", target_bir_lowering=False, debug=False, num_devices=NCORES
    )
    lhs = nc.dram_tensor("lhs", [BPC * K, N], BF16, kind="ExternalInput").ap()
    rhs = nc.dram_tensor("rhs", [BPC * K, M], BF16, kind="ExternalInput").ap()
    rowmin = nc.dram_tensor("rowmin", [BPC * PCH, NP], F32, kind="ExternalOutput").ap()
    # per (batch, g-chunk): row 0 = colstate final; rows 1..NGP = GpSimd
    # per-chunk column partials. Host max-combines.
    colmin = nc.dram_tensor(
        "colmin", [BPC * NG * (NGP + 1), FDV], BF16, kind="ExternalOutput"
    ).ap()

    with tile.TileContext(nc) as tc, ExitStack() as ctx:
        lr_pool = ctx.enter_context(tc.tile_pool(name="lr", bufs=2))
        col_pool = ctx.enter_context(tc.tile_pool(name="col", bufs=2 * NG))
        red_pool = ctx.enter_context(tc.tile_pool(name="red", bufs=8))
        gp_pool = ctx.enter_context(tc.tile_pool(name="gp", bufs=2 * NGP))
        d2_pool = ctx.enter_context(tc.tile_pool(name="d2", bufs=6))
        scr_pool = ctx.enter_context(tc.tile_pool(name="scr", bufs=3))
        acc_pool = ctx.enter_context(tc.tile_pool(name="acc", bufs=2))
        rp_pool = ctx.enter_context(tc.tile_pool(name="rp", bufs=2))
        psum_pool = ctx.enter_context(tc.tile_pool(name="psum", bufs=2, space="PSUM"))

        for i in range(BPC):
            L = lr_pool.tile([K, N], BF16, tag="L")
            nc.sync.dma_start(L[:], lhs[K * i : K * (i + 1), :])
            R = lr_pool.tile([K, M], BF16, tag="R")
            nc.sync.dma_start(R[:], rhs[K * i : K * (i + 1), :])

            colstate = [
                col_pool.tile([PCH, FDV], BF16, tag="cs", name=f"cs_{i}_{g}")
                for g in range(NG)
            ]
            rowpart = rp_pool.tile([PCH, NP * SEG], BF16, tag="rowpart")
            rowacc = acc_pool.tile([PCH, NP], F32, tag="rowacc")

            for p in range(NP):
                pg_tiles = []
                for g in range(NG):
                    # s = -d2 for this [PCH, FDV] chunk, via augmented matmul
                    ps = psum_pool.tile([PCH, FDV], F32, tag="ps")
                    for s in range(FDV // MMN):
                        nc.tensor.matmul(
                            ps[:, MMN * s : MMN * (s + 1)],
                            lhsT=L[:, PCH * p : PCH * (p + 1)],
                            rhs=R[:, FDV * g + MMN * s : FDV * g + MMN * (s + 1)],
                            start=True,
                            stop=True,
                        )
                    # stage to SBUF as bf16; p==0 lands directly in colstate
                    if p == 0:
                        dst = colstate[g]
                    elif p in GP_PS:
                        dst = gp_pool.tile([PCH, FDV], BF16, tag="gp")
                    else:
                        dst = d2_pool.tile([PCH, FDV], BF16, tag="d2")
                    nc.scalar.copy(dst[:], ps[:])
                    pg_tiles.append(dst)

                    # column pass (p==0 initialized by the staging copy)
                    if p in GP_PS:
                        csr = red_pool.tile(
                            [PCH, FDV], BF16, tag="csr", name=f"gp_{i}_{p}_{g}"
                        )
                        nc.gpsimd.partition_all_reduce(
                            csr[:], dst[:], channels=PCH,
                            reduce_op=bass_isa.ReduceOp.max,
                        )
                        row = (i * NG + g) * (NGP + 1) + 1 + GP_PS.index(p)
                        nc.sync.dma_start(colmin[row : row + 1, :], csr[0:1, :])
                    elif p > 0:
                        nc.vector.tensor_tensor(
                            out=colstate[g][:], in0=colstate[g][:], in1=dst[:], op=MAX
                        )

                # row pass: combine g chunks into scr[0:2048], then fold by
                # halves at 2x into successive scr segments; the last fold
                # (SEG wide) lands in rowpart.
                scr = scr_pool.tile([PCH, 2 * FDV], BF16, tag="scr")
                nc.vector.tensor_tensor(
                    out=scr[:, 0:FDV], in0=pg_tiles[0][:], in1=pg_tiles[1][:], op=MAX
                )
                src_off, w, pos = 0, FDV // 2, FDV
                while w >= SEG:
                    out_ap = (
                        rowpart[:, p * SEG : (p + 1) * SEG]
                        if w == SEG
                        else scr[:, pos : pos + w]
                    )
                    nc.vector.tensor_tensor(
                        out=out_ap,
                        in0=scr[:, src_off : src_off + w],
                        in1=scr[:, src_off + w : src_off + 2 * w],
                        op=MAX,
                    )
                    src_off = pos
                    pos += w
                    w //= 2

            # batch-final row fold: reduce each p's SEG-wide stub -> [PCH, NP]
            nc.vector.tensor_reduce(
                out=rowacc[:],
                in_=rowpart[:].rearrange("p (a b) -> p a b", b=SEG),
                axis=mybir.AxisListType.X,
                op=MAX,
            )
            nc.sync.dma_start(rowmin[PCH * i : PCH * (i + 1), :], rowacc[:])

            # colstate finals on GpSimd
            for g in range(NG):
                csr = red_pool.tile([PCH, FDV], BF16, tag="csr", name=f"csr_{i}_{g}")
                nc.gpsimd.partition_all_reduce(
                    csr[:], colstate[g][:], channels=PCH,
                    reduce_op=bass_isa.ReduceOp.max,
                )
                row = (i * NG + g) * (NGP + 1)
                nc.sync.dma_start(colmin[row : row + 1, :], csr[0:1, :])

    nc.compile()
    return nc


@lru_cache(maxsize=1)
def _get_program():
    return _build_program()


def _split3(x):
    """fp32 -> three bf16 terms whose sum matches x to ~2^-27 rel."""
    h = x.astype(NPBF16)
    r = x - h.astype(np.float32)
    m = r.astype(NPBF16)
    l = (r - m.astype(np.float32)).astype(NPBF16)
    return h, m, l


def _make_inputs(pred, gt):
    """Host-side packing of the K=24 bf16 split operands (for -d2), per core."""
    pred = np.ascontiguousarray(pred, dtype=np.float32)
    gt = np.ascontiguousarray(gt, dtype=np.float32)
    p2 = np.einsum("bnd,bnd->bn", pred, pred)
    g2 = np.einsum("bmd,bmd->bm", gt, gt)
    Lr, Rr = [], []
    for d in range(3):
        u = np.float32(2.0) * pred[:, :, d]  # +2 so the dot yields -d2
        v = gt[:, :, d]
        uh, um, ul = _split3(u)
        vh, vm, vl = _split3(v)
        # product pairs kept: hh, hm, mh, hl, lh, mm
        Lr += [uh, uh, um, uh, ul, um]
        Rr += [vh, vm, vh, vl, vh, vm]
    ph, pm, pl = _split3(-p2)
    gh, gm, gl = _split3(g2)
    ones_n = np.ones_like(p2, dtype=NPBF16)
    neg_n = -ones_n
    ones_m = np.ones_like(g2, dtype=NPBF16)
    Lr += [ph, pm, pl, neg_n, neg_n, neg_n]
    Rr += [ones_m, ones_m, ones_m, gh, gm, gl]
    lhs = np.stack(Lr, axis=1)  # [B, K, N] bf16
    rhs = np.stack(Rr, axis=1)  # [B, K, M] bf16
    in_maps = []
    for c in range(NCORES):
        sl = slice(c * BPC, (c + 1) * BPC)
        in_maps.append(
            {
                "lhs": np.ascontiguousarray(lhs[sl].reshape(BPC * K, N)),
                "rhs": np.ascontiguousarray(rhs[sl].reshape(BPC * K, M)),
            }
        )
    return in_maps


def _finish(results):
    # device values are maxes of -d2: negate back to d2 mins
    rowmins = -np.stack([r["rowmin"] for r in results]).astype(np.float64)
    colraw = np.stack(
        [np.asarray(r["colmin"]).astype(np.float32) for r in results]
    )  # [8, BPC*NG*(NGP+1), FDV]
    colmins = -(
        colraw.reshape(NCORES, BPC, NG, NGP + 1, FDV).max(axis=3)
    ).reshape(NCORES, BPC, M).astype(np.float64)
    ch2 = np.sqrt(np.maximum(rowmins, 1e-12)).mean()
    ch1 = np.sqrt(np.maximum(colmins, 1e-12)).mean()
    return np.asarray(ch1 + ch2, dtype=np.float32)


def kernel(pred, gt):
    nc = _get_program()
    in_maps = _make_inputs(pred, gt)
    res = run_bass_kernel_spmd(nc, in_maps, list(range(NCORES)))
    return _finish(res.results)


if __name__ == "__main__":
    rng = np.random.default_rng(0)
    pred = rng.standard_normal((B, N, 3), dtype=np.float32)
    gt = rng.standard_normal((B, M, 3), dtype=np.float32)
    print(kernel(pred, gt))
